# revision 24
# baseline (speedup 1.0000x reference)
"""Single-head attention (B=4, S=4096, E=1024, H=64) on 8 TRN2 NeuronCores.

Sharding: core c -> (batch b = c//2, sequence half h = c%2). No collectives:
each core receives the transposed bf16 x for its WHOLE batch row, laid out
own-half-first, computes K/V for the full 4096-key sequence plus Q for its
own 2048 queries, then runs attention and the output projection for its
queries. Softmax over keys is permutation invariant, so the own-first key
order needs no unpermute.

Matmuls are bf16 (fp8 was measured numerically dead for this problem: the
softmax is extremely peaked, Neff ~ 6, so e4m3 noise doesn't average out).
All large matmuls use full 128x128 stationary tiles (zero/junk padded) to
keep the PE at speed -- masked sub-tiles clock-gate the PE.

Softmax exp is split across two engines: the ACT engine computes exact Exp
for ~2/3 of the score tiles, and the DVE computes the rest with a
bf16 Schraudolph approximation (i16 = 23.083*s + 16249; bitcast to bf16
is 2^(logit*log2e) with ~2.3% sawtooth error; measured end-to-end rel err
~8e-3, budget 2e-2). x input lands via two HWDGE queues (sync + scalar)
to halve the descriptor-generation serialization.

Output projection uses the augmented-row trick: W_out carries b_out as row
64 and the bf16 context carries the softmax denominator in row 64, so
(ctx_aug.T @ W_out_aug) * recip(denom) applies scale and bias in one pass
(denom * recip == 1)."""

import sys

import numpy as np

for _p in ("/opt/trn_rl_repo",):
    if _p not in sys.path:
        sys.path.insert(0, _p)

from contextlib import ExitStack

import ml_dtypes

import concourse.bass as bass  # noqa: F401  (import keeps bass registered)
import concourse.mybir as mybir
import concourse.tile as tile
from concourse import bacc, masks
from concourse.bass_utils import run_bass_kernel_spmd

F32 = mybir.dt.float32
BF16 = mybir.dt.bfloat16
I16 = mybir.dt.int16
AF = mybir.ActivationFunctionType
ALU = mybir.AluOpType

B, S, E, H = 4, 4096, 1024, 64
SH = S // 2           # queries per core
N_CORES = 8
ET = E // 128         # 8 embedding tiles
FC = 512              # projection chunk (cols of the seq axis)
NCH = S // FC         # 8 chunks over the full sequence
ST = S // 128         # 32 kj tiles over the full sequence
QC = 1024             # query chunk (one PSUM ctx tile)
SCALE = 0.125         # 1/sqrt(H)
# Schraudolph bf16 exp: i16 = round(128*log2(e)*(SCALE*s) + 16256 - 7.4)
SCH_A = 128.0 * 1.4426950408889634 * SCALE
SCH_B = 16256.0 - 7.4


def _emit(nc, tc, xt, wkv, wq, bkv, bq, wo, out_ext):
    with ExitStack() as top:
        const = top.enter_context(tc.tile_pool(name="const", bufs=1))

        ident = const.tile([128, 128], BF16)
        masks.make_identity(nc, ident[:])

        # Weights + biases on the gpsimd SWDGE queue; x gets both HWDGE
        # queues to itself so nothing delays the projection chunks.
        wkv_sb = const.tile([128, ET * 128], BF16)
        nc.gpsimd.dma_start(wkv_sb[:], wkv[:, :])
        wq_sb = const.tile([128, ET * 128], BF16)
        nc.gpsimd.dma_start(wq_sb[:], wq[:, :])
        bkv_sb = const.tile([128, 1], F32)
        nc.gpsimd.dma_start(bkv_sb[:], bkv.unsqueeze(1))
        bq_sb = const.tile([64, 1], F32)
        nc.gpsimd.dma_start(bq_sb[:], bq.unsqueeze(1))

        # Persistent operands. x_sb holds the 8 e-tiles side by side.
        x_sb = const.tile([128, ET * S], BF16)
        k2 = const.tile([128, S], BF16)     # kT on 0:64, zeros on 64:128
        q2 = const.tile([128, SH], BF16)    # qT on 0:64, zeros on 64:128
        vt_sb = const.tile([128, S], BF16)  # vT on rows 64:128 (PSUM-aligned)
        v_aug = const.tile([128, ST * 128], BF16)
        wo_sb = const.tile([128, E], BF16)
        ones11 = const.tile([1, 1], BF16)

        # x over both HWDGE queues. Narrow first blocks so the first
        # projection chunk starts ASAP (per-queue transfers serialize, so a
        # 2048-wide first block would delay chunk 0 by ~7us); wide later
        # blocks amortize the ~0.6us descriptor generation per DMA.
        for f0, w in ((0, 512), (512, 512), (1024, 1024), (2048, 2048)):
            for e in range(ET):
                eng = nc.sync if e % 2 == 0 else nc.scalar
                eng.dma_start(
                    x_sb[:, e * S + f0 : e * S + f0 + w],
                    xt[e * 128 : (e + 1) * 128, f0 : f0 + w],
                )

        # big zero-fills on the Pool queue (free after the weight DMAs);
        # v_aug on DVE (first vtp copy needs it early); only the ones
        # column + junk cols of v_aug need initialization
        nc.gpsimd.memset(k2[64:128, :], 0.0)
        nc.gpsimd.memset(q2[64:128, :], 0.0)
        nc.gpsimd.memset(ones11[:], 1.0)
        v_aug_t = v_aug[:].rearrange("p (t c) -> p t c", c=128)
        nc.vector.memset(v_aug_t[:, :, 65:128], 0.0)
        nc.vector.memset(v_aug_t[:, :, 64:65], 1.0)
        nc.gpsimd.dma_start(wo_sb[:], wo[:, :])  # needed only in phase C

        # ---- Phase A + B interleaved ------------------------------------
        # Chunks 0-3 (own half, with Q) run first. Chunks 4-7 (other half,
        # K/V only) are then interleaved with the first 16 kj tiles'
        # scores+exp; their ctx matmuls are deferred (ex tiles parked in
        # SBUF) because PSUM can't hold the ctx accumulators until the
        # projection pools close. The second 16 kj drain two deferred ctx
        # tiles each, so the PE streams without phase gaps.
        def emit_chunk(c, mkvp, vtp, mqp=None):
            f0 = c * FC
            mkv = mkvp.tile([128, FC], F32)
            for e in range(ET):
                nc.tensor.matmul(
                    mkv[:],
                    wkv_sb[:, e * 128 : (e + 1) * 128],
                    x_sb[:, e * S + f0 : e * S + f0 + FC],
                    start=(e == 0), stop=(e == ET - 1),
                )
            if mqp is not None:  # own half: also project Q
                mq = mqp.tile([128, FC], F32)
                for e in range(ET):
                    nc.tensor.matmul(
                        mq[:],
                        wq_sb[:, e * 128 : (e + 1) * 128],
                        x_sb[:, e * S + f0 : e * S + f0 + FC],
                        start=(e == 0), stop=(e == ET - 1),
                    )
                nc.vector.tensor_scalar_add(
                    q2[0:64, f0 : f0 + FC], mq[0:64, :], bq_sb[:]
                )
            nc.vector.tensor_scalar_add(
                k2[0:64, f0 : f0 + FC], mkv[0:64, :], bkv_sb[0:64, :]
            )
            # vT staging on ACT: DVE is the phase-A co-bottleneck, ACT idles
            nc.scalar.add(
                vt_sb[64:128, f0 : f0 + FC], mkv[64:128, :], bkv_sb[64:128, :]
            )
            for t in range(FC // 128):
                kj = c * (FC // 128) + t
                vp = vtp.tile([128, 64], F32)
                nc.tensor.matmul(
                    vp[:],
                    vt_sb[64:128, kj * 128 : (kj + 1) * 128],
                    ident[64:128, 64:128],
                )
                nc.vector.tensor_copy(v_aug[:, kj * 128 : kj * 128 + 64], vp[:])

        with ExitStack() as pbc:
            expp = pbc.enter_context(tc.tile_pool(name="expp", bufs=24))
            sps = tc.alloc_tile_pool(name="sps", bufs=2, space="PSUM")

            pa = ExitStack()
            mkvp = pa.enter_context(tc.tile_pool(name="mkv", bufs=2, space="PSUM"))
            vtp = pa.enter_context(tc.tile_pool(name="vtp", bufs=1, space="PSUM"))
            with ExitStack() as pq:
                mqp = pq.enter_context(tc.tile_pool(name="mq", bufs=1, space="PSUM"))
                for c in range(4):
                    emit_chunk(c, mkvp, vtp, mqp)

            ex_store = {}

            def sc_exp(kj):
                lhs_k = k2[:, kj * 128 : (kj + 1) * 128]
                exs = []
                for qix in range(2):
                    q0 = qix * QC
                    sp = sps.tile([128, QC], F32, tag="sp")
                    for n in range(QC // 512):
                        nc.tensor.matmul(
                            sp[:, n * 512 : (n + 1) * 512],
                            lhs_k,
                            q2[:, q0 + n * 512 : q0 + (n + 1) * 512],
                        )
                    ex = expp.tile([128, QC], BF16)
                    # split the softmax exp: ACT gets 2 of every 3 tiles
                    # (exact), DVE the third (Schraudolph bf16 bitcast)
                    if (2 * kj + qix) % 3 == 2:
                        nc.vector.tensor_scalar(
                            ex[:].bitcast(I16),
                            sp[:],
                            SCH_A,
                            SCH_B,
                            op0=ALU.mult,
                            op1=ALU.add,
                        )
                    else:
                        nc.scalar.activation(ex[:], sp[:], AF.Exp, scale=SCALE)
                    exs.append(ex)
                ex_store[kj] = exs

            # other-half projection interleaved with the first 8 kj tiles'
            # scores+exp (their ctx is deferred -- the ctx accumulators
            # can't fit in PSUM until the projection pools close); scores
            # first, since their deps landed with chunks 0-3 while chunk
            # 4+ still waits on its x block
            for i in range(4):
                sc_exp(2 * i)
                sc_exp(2 * i + 1)
                emit_chunk(4 + i, mkvp, vtp)
            pa.close()

            cps = tc.alloc_tile_pool(name="cps", bufs=2, space="PSUM")
            ctxs = [
                cps.tile([128, QC], F32, tag="ctx", name=f"ctx{i}")
                for i in range(2)
            ]

            def emit_ctx(kj):
                lhs_v = v_aug[:, kj * 128 : (kj + 1) * 128]
                exs = ex_store.pop(kj)
                for qix in range(2):
                    for n in range(QC // 512):
                        nc.tensor.matmul(
                            ctxs[qix][:, n * 512 : (n + 1) * 512],
                            lhs_v,
                            exs[qix][:, n * 512 : (n + 1) * 512],
                            start=(kj == 0), stop=(kj == ST - 1),
                            skip_group_check=True,
                        )

            # drain the deferred ctx backlog two per new kj until the loop
            # reaches the steady lag-1 software pipeline
            backlog = list(range(8))
            for kj in range(8, ST):
                sc_exp(kj)
                backlog.append(kj)
                for _ in range(2):
                    if len(backlog) > 1:
                        emit_ctx(backlog.pop(0))
            while backlog:
                emit_ctx(backlog.pop(0))

            # ---- Phase C: output projection -----------------------------
            with ExitStack() as pc:
                ctxp = pc.enter_context(tc.tile_pool(name="ctxp", bufs=2))
                rsp = pc.enter_context(tc.tile_pool(name="rsp", bufs=4))
                outp = pc.enter_context(tc.tile_pool(name="outp", bufs=6))

                # drain ctx out of PSUM first so sps can release its banks
                # (LIFO pool stack) before the phase C PSUM pools open
                ctx16s, rs_rows = [], []
                for qix in range(2):
                    ctx16 = ctxp.tile([128, QC], BF16, tag="ctx16")
                    # rows 65:128 are exact zeros (v_aug junk cols are 0)
                    nc.vector.tensor_copy(ctx16[:], ctxs[qix][:])
                    ctx16s.append(ctx16)
                    rs_row = rsp.tile([1, QC], BF16, tag="rsrow")
                    nc.sync.dma_start(rs_row[:], ctx16[64:65, :])
                    rs_rows.append(rs_row)
                cps.release()
                sps.release()
                ops = pc.enter_context(tc.tile_pool(name="ops", bufs=3, space="PSUM"))
                rsps = pc.enter_context(tc.tile_pool(name="rsps", bufs=2, space="PSUM"))

                recips = []
                for qix in range(2):
                    rs_ps = rsps.tile([128, QC // 128], F32, tag="rsps")
                    for cc in range(QC // 128):
                        nc.tensor.matmul(
                            rs_ps[:, cc : cc + 1],
                            rs_rows[qix][0:1, cc * 128 : (cc + 1) * 128],
                            ones11[:],
                        )
                    recip = rsp.tile([128, QC // 128], F32, tag="recip")
                    nc.vector.reciprocal(recip[:], rs_ps[:])
                    recips.append(recip)

                # matmuls don't need recip -- emit them densely (keeps the
                # PE streaming so HAM holds full clock); scale-muls trail
                pend_mul = []
                for cc in range(QC // 128):
                    for qix in range(2):
                        ctx16, recip = ctx16s[qix], recips[qix]
                        out_sb = outp.tile([128, E], BF16)
                        for n in range(2):
                            op = ops.tile([128, 512], F32)
                            nc.tensor.matmul(
                                op[:],
                                ctx16[:, cc * 128 : (cc + 1) * 128],
                                wo_sb[:, n * 512 : (n + 1) * 512],
                            )
                            pend_mul.append((op, out_sb, qix, cc, n))
                        while len(pend_mul) > 2:
                            _emit_mul(nc, out_ext, recips, pend_mul.pop(0))
                while pend_mul:
                    _emit_mul(nc, out_ext, recips, pend_mul.pop(0))


def _emit_mul(nc, out_ext, recips, item):
    op, out_sb, qix, cc, n = item
    # Pool can't read PSUM; ACT is idle in phase C, so the recip-scale
    # alternates between DVE and ACT
    if (cc + n + qix) % 2 == 0:
        nc.vector.tensor_scalar_mul(
            out_sb[:, n * 512 : (n + 1) * 512], op[:], recips[qix][:, cc : cc + 1]
        )
    else:
        nc.scalar.mul(
            out_sb[:, n * 512 : (n + 1) * 512], op[:], recips[qix][:, cc : cc + 1]
        )
    if n == 1:
        nc.sync.dma_start(
            out_ext[qix * QC + cc * 128 : qix * QC + (cc + 1) * 128, :], out_sb[:]
        )


_NC = None


def _get_nc():
    global _NC
    if _NC is None:
        nc = bacc.Bacc("TRN2", target_bir_lowering=False, debug=False,
                       num_devices=N_CORES)
        xt = nc.dram_tensor("xt", [E, S], BF16, kind="ExternalInput").ap()
        wkv = nc.dram_tensor("wkv", [128, ET * 128], BF16, kind="ExternalInput").ap()
        wq = nc.dram_tensor("wq", [128, ET * 128], BF16, kind="ExternalInput").ap()
        bkv = nc.dram_tensor("bkv", [128], F32, kind="ExternalInput").ap()
        bq = nc.dram_tensor("bq", [64], F32, kind="ExternalInput").ap()
        wo = nc.dram_tensor("wo", [128, E], BF16, kind="ExternalInput").ap()
        out_ext = nc.dram_tensor("out", [SH, E], BF16, kind="ExternalOutput").ap()
        with tile.TileContext(nc) as tc:
            _emit(nc, tc, xt, wkv, wq, bkv, bq, wo, out_ext)
        nc.compile()
        _NC = nc
    return _NC


last_results = None
last_tmpdir = None


def kernel(x, W_qkv, b_qkv, W_out, b_out):
    nc = _get_nc()
    bf = ml_dtypes.bfloat16
    x = np.asarray(x, dtype=np.float32)
    Wq = np.asarray(W_qkv, dtype=np.float32)
    b1 = np.asarray(b_qkv, dtype=np.float32)

    wkv = np.empty((128, ET * 128), dtype=bf)
    wq_p = np.zeros((128, ET * 128), dtype=bf)
    for e in range(ET):
        wkv[:, e * 128 : e * 128 + 64] = Wq[e * 128 : (e + 1) * 128, 64:128]
        wkv[:, e * 128 + 64 : (e + 1) * 128] = Wq[e * 128 : (e + 1) * 128, 128:192]
        wq_p[:, e * 128 : e * 128 + 64] = Wq[e * 128 : (e + 1) * 128, 0:64]
    bkv = np.concatenate([b1[64:128], b1[128:192]]).astype(np.float32)
    bq = np.ascontiguousarray(b1[0:64])
    wo = np.zeros((128, E), dtype=bf)
    wo[0:64] = np.asarray(W_out, dtype=np.float32)
    wo[64] = np.asarray(b_out, dtype=np.float32)

    shared = {"wkv": wkv, "wq": wq_p, "bkv": bkv, "bq": bq, "wo": wo}
    in_maps = []
    for c in range(N_CORES):
        b, h = divmod(c, 2)
        xb = x[b]
        xt = np.empty((E, S), dtype=bf)
        xt[:, 0:SH] = xb[h * SH : (h + 1) * SH].T
        xt[:, SH:S] = xb[(1 - h) * SH : (2 - h) * SH].T
        in_maps.append({"xt": xt, **shared})

    import os
    import tempfile
    import time

    tmpdir = os.environ.get("ATTN_TRACE_DIR") or tempfile.mkdtemp(prefix="attn_trace_")
    res = None
    for attempt in range(3):
        try:
            res = run_bass_kernel_spmd(
                nc, in_maps, core_ids=list(range(N_CORES)), tmpdir=tmpdir
            )
            break
        except Exception:
            # transient NRT_EXEC_UNIT_UNRECOVERABLE has been observed on a
            # first attempt; a clean retry recovers
            if attempt == 2:
                raise
            time.sleep(2.0)
    global last_results, last_tmpdir
    last_results = res
    last_tmpdir = tmpdir

    out = np.empty((B, S, E), dtype=np.float32)
    for c in range(N_CORES):
        b, h = divmod(c, 2)
        out[b, h * SH : (h + 1) * SH] = res.results[c]["out"].astype(np.float32)
    return out


# revision 26
# speedup vs baseline: 1.0209x; 1.0209x over previous
"""Single-head attention (B=4, S=4096, E=1024, H=64) on 8 TRN2 NeuronCores.

Sharding: core c -> (batch b = c//2, sequence half h = c%2). No collectives:
each core receives the transposed bf16 x for its WHOLE batch row, laid out
own-half-first, computes K/V for the full 4096-key sequence plus Q for its
own 2048 queries, then runs attention and the output projection for its
queries. Softmax over keys is permutation invariant, so the own-first key
order needs no unpermute.

Matmuls are bf16 (fp8 was measured numerically dead for this problem: the
softmax is extremely peaked, Neff ~ 6, so e4m3 noise doesn't average out).
All large matmuls use full 128x128 stationary tiles (zero/junk padded) to
keep the PE at speed -- masked sub-tiles clock-gate the PE.

Softmax exp is split across two engines: the ACT engine computes exact Exp
for ~2/3 of the score tiles, and the DVE computes the rest with a
bf16 Schraudolph approximation (i16 = 23.083*s + 16249; bitcast to bf16
is 2^(logit*log2e) with ~2.3% sawtooth error; measured end-to-end rel err
~8e-3, budget 2e-2). x input lands via two HWDGE queues (sync + scalar)
to halve the descriptor-generation serialization.

Output projection uses the augmented-row trick: W_out carries b_out as row
64 and the bf16 context carries the softmax denominator in row 64, so
(ctx_aug.T @ W_out_aug) * recip(denom) applies scale and bias in one pass
(denom * recip == 1)."""

import sys

import numpy as np

for _p in ("/opt/trn_rl_repo",):
    if _p not in sys.path:
        sys.path.insert(0, _p)

from contextlib import ExitStack

import ml_dtypes

import concourse.bass as bass  # noqa: F401  (import keeps bass registered)
import concourse.mybir as mybir
import concourse.tile as tile
from concourse import bacc, masks
from concourse.bass_utils import run_bass_kernel_spmd

F32 = mybir.dt.float32
BF16 = mybir.dt.bfloat16
I16 = mybir.dt.int16
AF = mybir.ActivationFunctionType
ALU = mybir.AluOpType

B, S, E, H = 4, 4096, 1024, 64
SH = S // 2           # queries per core
N_CORES = 8
ET = E // 128         # 8 embedding tiles
FC = 512              # projection chunk (cols of the seq axis)
NCH = S // FC         # 8 chunks over the full sequence
ST = S // 128         # 32 kj tiles over the full sequence
QC = 1024             # query chunk (one PSUM ctx tile)
SCALE = 0.125         # 1/sqrt(H)
# Schraudolph bf16 exp: i16 = round(128*log2(e)*(SCALE*s) + 16256 - 7.4)
SCH_A = 128.0 * 1.4426950408889634 * SCALE
SCH_B = 16256.0 - 7.4


def _emit(nc, tc, xt, wkv, wq, bkv, bq, wo, out_ext):
    with ExitStack() as top:
        const = top.enter_context(tc.tile_pool(name="const", bufs=1))

        ident = const.tile([128, 128], BF16)
        masks.make_identity(nc, ident[:])

        # Weights + biases on the gpsimd SWDGE queue; x gets both HWDGE
        # queues to itself so nothing delays the projection chunks.
        wkv_sb = const.tile([128, ET * 128], BF16)
        nc.gpsimd.dma_start(wkv_sb[:], wkv[:, :])
        wq_sb = const.tile([128, ET * 128], BF16)
        nc.gpsimd.dma_start(wq_sb[:], wq[:, :])
        bkv_sb = const.tile([128, 1], F32)
        nc.gpsimd.dma_start(bkv_sb[:], bkv.unsqueeze(1))
        bq_sb = const.tile([64, 1], F32)
        nc.gpsimd.dma_start(bq_sb[:], bq.unsqueeze(1))

        # Persistent operands. x_sb holds the 8 e-tiles side by side.
        x_sb = const.tile([128, ET * S], BF16)
        k2 = const.tile([128, S], BF16)     # kT on 0:64, zeros on 64:128
        q2 = const.tile([128, SH], BF16)    # qT on 0:64, zeros on 64:128
        vt_sb = const.tile([128, S], BF16)  # vT on rows 64:128 (PSUM-aligned)
        v_aug = const.tile([128, ST * 128], BF16)
        wo_sb = const.tile([128, E], BF16)
        ones11 = const.tile([1, 1], BF16)

        # x over both HWDGE queues. Narrow first blocks so the first
        # projection chunk starts ASAP (per-queue transfers serialize, so a
        # 2048-wide first block would delay chunk 0 by ~7us); wide later
        # blocks amortize the ~0.6us descriptor generation per DMA.
        for f0, w in ((0, 512), (512, 512), (1024, 1024), (2048, 2048)):
            for e in range(ET):
                eng = nc.sync if e % 2 == 0 else nc.scalar
                eng.dma_start(
                    x_sb[:, e * S + f0 : e * S + f0 + w],
                    xt[e * 128 : (e + 1) * 128, f0 : f0 + w],
                )

        # big zero-fills on the Pool queue (free after the weight DMAs);
        # v_aug on DVE (first vtp copy needs it early); only the ones
        # column + junk cols of v_aug need initialization
        nc.gpsimd.memset(k2[64:128, :], 0.0)
        nc.gpsimd.memset(q2[64:128, :], 0.0)
        nc.gpsimd.memset(ones11[:], 1.0)
        v_aug_t = v_aug[:].rearrange("p (t c) -> p t c", c=128)
        nc.vector.memset(v_aug_t[:, :, 65:128], 0.0)
        nc.vector.memset(v_aug_t[:, :, 64:65], 1.0)
        nc.gpsimd.dma_start(wo_sb[:], wo[:, :])  # needed only in phase C

        # ---- Phase A + B interleaved ------------------------------------
        # Chunks 0-3 (own half, with Q) run first. Chunks 4-7 (other half,
        # K/V only) are then interleaved with the first 16 kj tiles'
        # scores+exp; their ctx matmuls are deferred (ex tiles parked in
        # SBUF) because PSUM can't hold the ctx accumulators until the
        # projection pools close. The second 16 kj drain two deferred ctx
        # tiles each, so the PE streams without phase gaps.
        def emit_chunk(c, mkvp, mqp=None):
            f0 = c * FC
            mkv = mkvp.tile([128, FC], F32)
            for e in range(ET):
                nc.tensor.matmul(
                    mkv[:],
                    wkv_sb[:, e * 128 : (e + 1) * 128],
                    x_sb[:, e * S + f0 : e * S + f0 + FC],
                    start=(e == 0), stop=(e == ET - 1),
                )
            if mqp is not None:  # own half: also project Q
                mq = mqp.tile([128, FC], F32)
                for e in range(ET):
                    nc.tensor.matmul(
                        mq[:],
                        wq_sb[:, e * 128 : (e + 1) * 128],
                        x_sb[:, e * S + f0 : e * S + f0 + FC],
                        start=(e == 0), stop=(e == ET - 1),
                    )
                nc.vector.tensor_scalar_add(
                    q2[0:64, f0 : f0 + FC], mq[0:64, :], bq_sb[:]
                )
            nc.vector.tensor_scalar_add(
                k2[0:64, f0 : f0 + FC], mkv[0:64, :], bkv_sb[0:64, :]
            )
            # vT staging on ACT: DVE is the phase-A co-bottleneck, ACT idles
            nc.scalar.add(
                vt_sb[64:128, f0 : f0 + FC], mkv[64:128, :], bkv_sb[64:128, :]
            )

        def emit_vps(c, vtp):
            # V transposes for chunk c, emitted one chunk late so the PE
            # queue never stalls on the DVE v_aug copy draining a vp slot
            for t in range(FC // 128):
                kj = c * (FC // 128) + t
                vp = vtp.tile([128, 64], F32)
                nc.tensor.matmul(
                    vp[:],
                    vt_sb[64:128, kj * 128 : (kj + 1) * 128],
                    ident[64:128, 64:128],
                )
                nc.vector.tensor_copy(v_aug[:, kj * 128 : kj * 128 + 64], vp[:])

        with ExitStack() as pbc:
            expp = pbc.enter_context(tc.tile_pool(name="expp", bufs=24))
            sps = tc.alloc_tile_pool(name="sps", bufs=2, space="PSUM")

            pa = ExitStack()
            mkvp = pa.enter_context(tc.tile_pool(name="mkv", bufs=2, space="PSUM"))
            with ExitStack() as pq:
                mqp = pq.enter_context(tc.tile_pool(name="mq", bufs=1, space="PSUM"))
                vtp0 = pq.enter_context(tc.tile_pool(name="vtp0", bufs=1, space="PSUM"))
                for c in range(4):
                    emit_chunk(c, mkvp, mqp)
                    if c > 0:
                        emit_vps(c - 1, vtp0)

            vtp = pa.enter_context(tc.tile_pool(name="vtp", bufs=2, space="PSUM"))
            ex_store = {}

            def sc_exp(kj):
                lhs_k = k2[:, kj * 128 : (kj + 1) * 128]
                exs = []
                for qix in range(2):
                    q0 = qix * QC
                    sp = sps.tile([128, QC], F32, tag="sp")
                    for n in range(QC // 512):
                        nc.tensor.matmul(
                            sp[:, n * 512 : (n + 1) * 512],
                            lhs_k,
                            q2[:, q0 + n * 512 : q0 + (n + 1) * 512],
                        )
                    ex = expp.tile([128, QC], BF16)
                    # split the softmax exp: ACT gets 2 of every 3 tiles
                    # (exact), DVE the third (Schraudolph bf16 bitcast)
                    if (2 * kj + qix) % 3 == 2:
                        nc.vector.tensor_scalar(
                            ex[:].bitcast(I16),
                            sp[:],
                            SCH_A,
                            SCH_B,
                            op0=ALU.mult,
                            op1=ALU.add,
                        )
                    else:
                        nc.scalar.activation(ex[:], sp[:], AF.Exp, scale=SCALE)
                    exs.append(ex)
                ex_store[kj] = exs

            # other-half projection interleaved with the first 8 kj tiles'
            # scores+exp (their ctx is deferred -- the ctx accumulators
            # can't fit in PSUM until the projection pools close); scores
            # first, since their deps landed with chunks 0-3 while chunk
            # 4+ still waits on its x block
            for i in range(4):
                sc_exp(2 * i)
                sc_exp(2 * i + 1)
                emit_chunk(4 + i, mkvp)
                emit_vps(3 + i, vtp)
            emit_vps(7, vtp)
            pa.close()

            cps = tc.alloc_tile_pool(name="cps", bufs=2, space="PSUM")
            ctxs = [
                cps.tile([128, QC], F32, tag="ctx", name=f"ctx{i}")
                for i in range(2)
            ]

            def emit_ctx(kj):
                lhs_v = v_aug[:, kj * 128 : (kj + 1) * 128]
                exs = ex_store.pop(kj)
                for qix in range(2):
                    for n in range(QC // 512):
                        nc.tensor.matmul(
                            ctxs[qix][:, n * 512 : (n + 1) * 512],
                            lhs_v,
                            exs[qix][:, n * 512 : (n + 1) * 512],
                            start=(kj == 0), stop=(kj == ST - 1),
                            skip_group_check=True,
                        )

            # drain the deferred ctx backlog two per new kj until the loop
            # reaches the steady lag-1 software pipeline
            backlog = list(range(8))
            for kj in range(8, ST):
                sc_exp(kj)
                backlog.append(kj)
                for _ in range(2):
                    if len(backlog) > 1:
                        emit_ctx(backlog.pop(0))
            while backlog:
                emit_ctx(backlog.pop(0))

            # ---- Phase C: output projection -----------------------------
            with ExitStack() as pc:
                ctxp = pc.enter_context(tc.tile_pool(name="ctxp", bufs=2))
                rsp = pc.enter_context(tc.tile_pool(name="rsp", bufs=4))
                outp = pc.enter_context(tc.tile_pool(name="outp", bufs=6))

                # drain ctx out of PSUM first so sps can release its banks
                # (LIFO pool stack) before the phase C PSUM pools open
                ctx16s, rs_rows = [], []
                for qix in range(2):
                    ctx16 = ctxp.tile([128, QC], BF16, tag="ctx16")
                    # rows 65:128 are exact zeros (v_aug junk cols are 0)
                    nc.vector.tensor_copy(ctx16[:], ctxs[qix][:])
                    ctx16s.append(ctx16)
                    rs_row = rsp.tile([1, QC], BF16, tag="rsrow")
                    nc.sync.dma_start(rs_row[:], ctx16[64:65, :])
                    rs_rows.append(rs_row)
                cps.release()
                sps.release()
                ops = pc.enter_context(tc.tile_pool(name="ops", bufs=3, space="PSUM"))
                rsps = pc.enter_context(tc.tile_pool(name="rsps", bufs=2, space="PSUM"))

                recips = []
                for qix in range(2):
                    rs_ps = rsps.tile([128, QC // 128], F32, tag="rsps")
                    for cc in range(QC // 128):
                        nc.tensor.matmul(
                            rs_ps[:, cc : cc + 1],
                            rs_rows[qix][0:1, cc * 128 : (cc + 1) * 128],
                            ones11[:],
                        )
                    recip = rsp.tile([128, QC // 128], F32, tag="recip")
                    nc.vector.reciprocal(recip[:], rs_ps[:])
                    recips.append(recip)

                # matmuls don't need recip -- emit them densely (keeps the
                # PE streaming so HAM holds full clock); scale-muls trail
                pend_mul = []
                for cc in range(QC // 128):
                    for qix in range(2):
                        ctx16, recip = ctx16s[qix], recips[qix]
                        out_sb = outp.tile([128, E], BF16)
                        for n in range(2):
                            op = ops.tile([128, 512], F32)
                            nc.tensor.matmul(
                                op[:],
                                ctx16[:, cc * 128 : (cc + 1) * 128],
                                wo_sb[:, n * 512 : (n + 1) * 512],
                            )
                            pend_mul.append((op, out_sb, qix, cc, n))
                        while len(pend_mul) > 2:
                            _emit_mul(nc, out_ext, recips, pend_mul.pop(0))
                while pend_mul:
                    _emit_mul(nc, out_ext, recips, pend_mul.pop(0))


def _emit_mul(nc, out_ext, recips, item):
    op, out_sb, qix, cc, n = item
    # Pool can't read PSUM; ACT is idle in phase C, so the recip-scale
    # alternates between DVE and ACT
    if (cc + n + qix) % 2 == 0:
        nc.vector.tensor_scalar_mul(
            out_sb[:, n * 512 : (n + 1) * 512], op[:], recips[qix][:, cc : cc + 1]
        )
    else:
        nc.scalar.mul(
            out_sb[:, n * 512 : (n + 1) * 512], op[:], recips[qix][:, cc : cc + 1]
        )
    if n == 1:
        nc.sync.dma_start(
            out_ext[qix * QC + cc * 128 : qix * QC + (cc + 1) * 128, :], out_sb[:]
        )


_NC = None


def _get_nc():
    global _NC
    if _NC is None:
        nc = bacc.Bacc("TRN2", target_bir_lowering=False, debug=False,
                       num_devices=N_CORES)
        xt = nc.dram_tensor("xt", [E, S], BF16, kind="ExternalInput").ap()
        wkv = nc.dram_tensor("wkv", [128, ET * 128], BF16, kind="ExternalInput").ap()
        wq = nc.dram_tensor("wq", [128, ET * 128], BF16, kind="ExternalInput").ap()
        bkv = nc.dram_tensor("bkv", [128], F32, kind="ExternalInput").ap()
        bq = nc.dram_tensor("bq", [64], F32, kind="ExternalInput").ap()
        wo = nc.dram_tensor("wo", [128, E], BF16, kind="ExternalInput").ap()
        out_ext = nc.dram_tensor("out", [SH, E], BF16, kind="ExternalOutput").ap()
        with tile.TileContext(nc) as tc:
            _emit(nc, tc, xt, wkv, wq, bkv, bq, wo, out_ext)
        nc.compile()
        _NC = nc
    return _NC


last_results = None
last_tmpdir = None


def kernel(x, W_qkv, b_qkv, W_out, b_out):
    nc = _get_nc()
    bf = ml_dtypes.bfloat16
    x = np.asarray(x, dtype=np.float32)
    Wq = np.asarray(W_qkv, dtype=np.float32)
    b1 = np.asarray(b_qkv, dtype=np.float32)

    wkv = np.empty((128, ET * 128), dtype=bf)
    wq_p = np.zeros((128, ET * 128), dtype=bf)
    for e in range(ET):
        wkv[:, e * 128 : e * 128 + 64] = Wq[e * 128 : (e + 1) * 128, 64:128]
        wkv[:, e * 128 + 64 : (e + 1) * 128] = Wq[e * 128 : (e + 1) * 128, 128:192]
        wq_p[:, e * 128 : e * 128 + 64] = Wq[e * 128 : (e + 1) * 128, 0:64]
    bkv = np.concatenate([b1[64:128], b1[128:192]]).astype(np.float32)
    bq = np.ascontiguousarray(b1[0:64])
    wo = np.zeros((128, E), dtype=bf)
    wo[0:64] = np.asarray(W_out, dtype=np.float32)
    wo[64] = np.asarray(b_out, dtype=np.float32)

    shared = {"wkv": wkv, "wq": wq_p, "bkv": bkv, "bq": bq, "wo": wo}
    in_maps = []
    for c in range(N_CORES):
        b, h = divmod(c, 2)
        xb = x[b]
        xt = np.empty((E, S), dtype=bf)
        xt[:, 0:SH] = xb[h * SH : (h + 1) * SH].T
        xt[:, SH:S] = xb[(1 - h) * SH : (2 - h) * SH].T
        in_maps.append({"xt": xt, **shared})

    import os
    import tempfile
    import time

    tmpdir = os.environ.get("ATTN_TRACE_DIR") or tempfile.mkdtemp(prefix="attn_trace_")
    res = None
    for attempt in range(3):
        try:
            res = run_bass_kernel_spmd(
                nc, in_maps, core_ids=list(range(N_CORES)), tmpdir=tmpdir
            )
            break
        except Exception:
            # transient NRT_EXEC_UNIT_UNRECOVERABLE has been observed on a
            # first attempt; a clean retry recovers
            if attempt == 2:
                raise
            time.sleep(2.0)
    global last_results, last_tmpdir
    last_results = res
    last_tmpdir = tmpdir

    out = np.empty((B, S, E), dtype=np.float32)
    for c in range(N_CORES):
        b, h = divmod(c, 2)
        out[b, h * SH : (h + 1) * SH] = res.results[c]["out"].astype(np.float32)
    return out


# revision 29
# speedup vs baseline: 1.0929x; 1.0706x over previous
"""Single-head attention (B=4, S=4096, E=1024, H=64) on 8 TRN2 NeuronCores.

Sharding: core c -> (batch b = c//2, sequence half h = c%2). No collectives:
each core receives the transposed bf16 x for its WHOLE batch row, laid out
own-half-first, computes K/V for the full 4096-key sequence plus Q for its
own 2048 queries, then runs attention and the output projection for its
queries. Softmax over keys is permutation invariant, so the own-first key
order needs no unpermute.

Matmuls are bf16 (fp8 was measured numerically dead for this problem: the
softmax is extremely peaked, Neff ~ 6, so e4m3 noise doesn't average out).
All large matmuls use full 128x128 stationary tiles (zero/junk padded) to
keep the PE at speed -- masked sub-tiles clock-gate the PE.

Softmax exp is split across two engines: the ACT engine computes exact Exp
for ~2/3 of the score tiles, and the DVE computes the rest with a
bf16 Schraudolph approximation (i16 = 23.083*s + 16249; bitcast to bf16
is 2^(logit*log2e) with ~2.3% sawtooth error; measured end-to-end rel err
~8e-3, budget 2e-2). x input lands via two HWDGE queues (sync + scalar)
to halve the descriptor-generation serialization.

Output projection uses the augmented-row trick: W_out carries b_out as row
64 and the bf16 context carries the softmax denominator in row 64, so
(ctx_aug.T @ W_out_aug) * recip(denom) applies scale and bias in one pass
(denom * recip == 1)."""

import sys

import numpy as np

for _p in ("/opt/trn_rl_repo",):
    if _p not in sys.path:
        sys.path.insert(0, _p)

from contextlib import ExitStack

import ml_dtypes

import concourse.bass as bass  # noqa: F401  (import keeps bass registered)
import concourse.mybir as mybir
import concourse.tile as tile
from concourse import bacc, masks
from concourse.bass_utils import run_bass_kernel_spmd

F32 = mybir.dt.float32
BF16 = mybir.dt.bfloat16
I16 = mybir.dt.int16
AF = mybir.ActivationFunctionType
ALU = mybir.AluOpType

B, S, E, H = 4, 4096, 1024, 64
SH = S // 2           # queries per core
N_CORES = 8
ET = E // 128         # 8 embedding tiles
FC = 512              # projection chunk (cols of the seq axis)
NCH = S // FC         # 8 chunks over the full sequence
ST = S // 128         # 32 kj tiles over the full sequence
QC = 1024             # query chunk (one PSUM ctx tile)
SCALE = 0.125         # 1/sqrt(H)
# Schraudolph bf16 exp: i16 = round(128*log2(e)*(SCALE*s) + 16256 - 7.4)
SCH_A = 128.0 * 1.4426950408889634 * SCALE
SCH_B = 16256.0 - 7.4


def _emit(nc, tc, xt, wkv, wq, bkv, bq, wo, out_ext):
    with ExitStack() as top:
        const = top.enter_context(tc.tile_pool(name="const", bufs=1))

        # Weights + biases on the gpsimd SWDGE queue; x gets both HWDGE
        # queues to itself so nothing delays the projection chunks.
        wkv_sb = const.tile([128, ET * 128], BF16)
        nc.gpsimd.dma_start(wkv_sb[:], wkv[:, :])
        wq_sb = const.tile([128, ET * 128], BF16)
        nc.gpsimd.dma_start(wq_sb[:], wq[:, :])
        bkv_sb = const.tile([128, 1], F32)
        nc.gpsimd.dma_start(bkv_sb[:], bkv.unsqueeze(1))
        bq_sb = const.tile([64, 1], F32)
        nc.gpsimd.dma_start(bq_sb[:], bq.unsqueeze(1))

        # Persistent operands. x_sb holds the 8 e-tiles side by side.
        x_sb = const.tile([128, ET * S], BF16)
        k2 = const.tile([128, S], BF16)     # kT on 0:64, zeros on 64:128
        q2 = const.tile([128, SH], BF16)    # qT on 0:64, zeros on 64:128
        vt_sb = const.tile([128, S], BF16)  # vT on rows 64:128 (PSUM-aligned)
        v_aug = const.tile([128, ST * 128], BF16)
        wo_sb = const.tile([128, E], BF16)
        ones11 = const.tile([1, 1], BF16)

        # x over both HWDGE queues. Narrow first blocks so the first
        # projection chunk starts ASAP (per-queue transfers serialize, so a
        # 2048-wide first block would delay chunk 0 by ~7us); wide later
        # blocks amortize the ~0.6us descriptor generation per DMA.
        for f0, w in ((0, 512), (512, 512), (1024, 1024), (2048, 2048)):
            for e in range(ET):
                eng = nc.sync if e % 2 == 0 else nc.scalar
                eng.dma_start(
                    x_sb[:, e * S + f0 : e * S + f0 + w],
                    xt[e * 128 : (e + 1) * 128, f0 : f0 + w],
                )

        # big zero-fills on the Pool queue (free after the weight DMAs);
        # v_aug on DVE (first vtp copy needs it early); only the ones
        # column + junk cols of v_aug need initialization
        nc.gpsimd.memset(k2[64:128, :], 0.0)
        nc.gpsimd.memset(q2[64:128, :], 0.0)
        nc.gpsimd.memset(ones11[:], 1.0)
        v_aug_t = v_aug[:].rearrange("p (t c) -> p t c", c=128)
        nc.vector.memset(v_aug_t[:, :, 65:128], 0.0)
        nc.vector.memset(v_aug_t[:, :, 64:65], 1.0)
        nc.gpsimd.dma_start(wo_sb[:], wo[:, :])  # needed only in phase C

        # ---- Phase A + B interleaved ------------------------------------
        # Chunks 0-3 (own half, with Q) run first. Chunks 4-7 (other half,
        # K/V only) are then interleaved with the first 16 kj tiles'
        # scores+exp; their ctx matmuls are deferred (ex tiles parked in
        # SBUF) because PSUM can't hold the ctx accumulators until the
        # projection pools close. The second 16 kj drain two deferred ctx
        # tiles each, so the PE streams without phase gaps.
        def emit_chunk(c, mkvp, mqp=None):
            f0 = c * FC
            mkv = mkvp.tile([128, FC], F32)
            for e in range(ET):
                nc.tensor.matmul(
                    mkv[:],
                    wkv_sb[:, e * 128 : (e + 1) * 128],
                    x_sb[:, e * S + f0 : e * S + f0 + FC],
                    start=(e == 0), stop=(e == ET - 1),
                )
            if mqp is not None:  # own half: also project Q
                mq = mqp.tile([128, FC], F32)
                for e in range(ET):
                    nc.tensor.matmul(
                        mq[:],
                        wq_sb[:, e * 128 : (e + 1) * 128],
                        x_sb[:, e * S + f0 : e * S + f0 + FC],
                        start=(e == 0), stop=(e == ET - 1),
                    )
                nc.vector.tensor_scalar_add(
                    q2[0:64, f0 : f0 + FC], mq[0:64, :], bq_sb[:]
                )
            nc.vector.tensor_scalar_add(
                k2[0:64, f0 : f0 + FC], mkv[0:64, :], bkv_sb[0:64, :]
            )
            # vT staging on ACT: DVE is the phase-A co-bottleneck, ACT idles
            nc.scalar.add(
                vt_sb[64:128, f0 : f0 + FC], mkv[64:128, :], bkv_sb[64:128, :]
            )
            # V transposed into v_aug by the DMA XBAR -- costs no PE or DVE
            # time, and the sync queue is idle mid-kernel
            for t in range(FC // 128):
                kj = c * (FC // 128) + t
                nc.sync.dma_start_transpose(
                    v_aug[:, kj * 128 : kj * 128 + 64],
                    vt_sb[64:128, kj * 128 : (kj + 1) * 128],
                )

        with ExitStack() as pbc:
            expp = pbc.enter_context(tc.tile_pool(name="expp", bufs=24))
            sps = tc.alloc_tile_pool(name="sps", bufs=2, space="PSUM")

            pa = ExitStack()
            mkvp = pa.enter_context(tc.tile_pool(name="mkv", bufs=2, space="PSUM"))
            with ExitStack() as pq:
                mqp = pq.enter_context(tc.tile_pool(name="mq", bufs=2, space="PSUM"))
                for c in range(4):
                    emit_chunk(c, mkvp, mqp)

            ex_store = {}

            def sc_exp(kj):
                lhs_k = k2[:, kj * 128 : (kj + 1) * 128]
                exs = []
                for qix in range(2):
                    q0 = qix * QC
                    sp = sps.tile([128, QC], F32, tag="sp")
                    for n in range(QC // 512):
                        nc.tensor.matmul(
                            sp[:, n * 512 : (n + 1) * 512],
                            lhs_k,
                            q2[:, q0 + n * 512 : q0 + (n + 1) * 512],
                        )
                    ex = expp.tile([128, QC], BF16)
                    # split the softmax exp: ACT gets 2 of every 3 tiles
                    # (exact), DVE the third (Schraudolph bf16 bitcast)
                    if (2 * kj + qix) % 3 == 2:
                        nc.vector.tensor_scalar(
                            ex[:].bitcast(I16),
                            sp[:],
                            SCH_A,
                            SCH_B,
                            op0=ALU.mult,
                            op1=ALU.add,
                        )
                    else:
                        nc.scalar.activation(ex[:], sp[:], AF.Exp, scale=SCALE)
                    exs.append(ex)
                ex_store[kj] = exs

            # other-half projection interleaved with the first 8 kj tiles'
            # scores+exp (their ctx is deferred -- the ctx accumulators
            # can't fit in PSUM until the projection pools close); scores
            # first, since their deps landed with chunks 0-3 while chunk
            # 4+ still waits on its x block
            for i in range(4):
                sc_exp(2 * i)
                sc_exp(2 * i + 1)
                emit_chunk(4 + i, mkvp)
            pa.close()

            cps = tc.alloc_tile_pool(name="cps", bufs=2, space="PSUM")
            ctxs = [
                cps.tile([128, QC], F32, tag="ctx", name=f"ctx{i}")
                for i in range(2)
            ]

            def emit_ctx(kj):
                lhs_v = v_aug[:, kj * 128 : (kj + 1) * 128]
                exs = ex_store.pop(kj)
                for qix in range(2):
                    for n in range(QC // 512):
                        nc.tensor.matmul(
                            ctxs[qix][:, n * 512 : (n + 1) * 512],
                            lhs_v,
                            exs[qix][:, n * 512 : (n + 1) * 512],
                            start=(kj == 0), stop=(kj == ST - 1),
                            skip_group_check=True,
                        )

            # drain the deferred ctx backlog two per new kj until the loop
            # reaches the steady lag-1 software pipeline
            backlog = list(range(8))
            for kj in range(8, ST):
                sc_exp(kj)
                backlog.append(kj)
                for _ in range(2):
                    if len(backlog) > 1:
                        emit_ctx(backlog.pop(0))
            while backlog:
                emit_ctx(backlog.pop(0))

            # ---- Phase C: output projection -----------------------------
            with ExitStack() as pc:
                ctxp = pc.enter_context(tc.tile_pool(name="ctxp", bufs=2))
                rsp = pc.enter_context(tc.tile_pool(name="rsp", bufs=4))
                outp = pc.enter_context(tc.tile_pool(name="outp", bufs=6))

                # drain ctx out of PSUM first so sps can release its banks
                # (LIFO pool stack) before the phase C PSUM pools open
                ctx16s, rs_rows = [], []
                for qix in range(2):
                    ctx16 = ctxp.tile([128, QC], BF16, tag="ctx16")
                    # rows 65:128 are exact zeros (v_aug junk cols are 0)
                    nc.vector.tensor_copy(ctx16[:], ctxs[qix][:])
                    ctx16s.append(ctx16)
                    rs_row = rsp.tile([1, QC], BF16, tag="rsrow")
                    nc.sync.dma_start(rs_row[:], ctx16[64:65, :])
                    rs_rows.append(rs_row)
                cps.release()
                sps.release()
                ops = pc.enter_context(tc.tile_pool(name="ops", bufs=3, space="PSUM"))
                rsps = pc.enter_context(tc.tile_pool(name="rsps", bufs=2, space="PSUM"))

                recips = []
                for qix in range(2):
                    rs_ps = rsps.tile([128, QC // 128], F32, tag="rsps")
                    for cc in range(QC // 128):
                        nc.tensor.matmul(
                            rs_ps[:, cc : cc + 1],
                            rs_rows[qix][0:1, cc * 128 : (cc + 1) * 128],
                            ones11[:],
                        )
                    recip = rsp.tile([128, QC // 128], F32, tag="recip")
                    nc.vector.reciprocal(recip[:], rs_ps[:])
                    recips.append(recip)

                # matmuls don't need recip -- emit them densely (keeps the
                # PE streaming so HAM holds full clock); scale-muls trail
                pend_mul = []
                for cc in range(QC // 128):
                    for qix in range(2):
                        ctx16, recip = ctx16s[qix], recips[qix]
                        out_sb = outp.tile([128, E], BF16)
                        for n in range(2):
                            op = ops.tile([128, 512], F32)
                            nc.tensor.matmul(
                                op[:],
                                ctx16[:, cc * 128 : (cc + 1) * 128],
                                wo_sb[:, n * 512 : (n + 1) * 512],
                            )
                            pend_mul.append((op, out_sb, qix, cc, n))
                        while len(pend_mul) > 2:
                            _emit_mul(nc, out_ext, recips, pend_mul.pop(0))
                while pend_mul:
                    _emit_mul(nc, out_ext, recips, pend_mul.pop(0))


def _emit_mul(nc, out_ext, recips, item):
    op, out_sb, qix, cc, n = item
    # Pool can't read PSUM; ACT is idle in phase C, so the recip-scale
    # alternates between DVE and ACT
    if (cc + n + qix) % 2 == 0:
        nc.vector.tensor_scalar_mul(
            out_sb[:, n * 512 : (n + 1) * 512], op[:], recips[qix][:, cc : cc + 1]
        )
    else:
        nc.scalar.mul(
            out_sb[:, n * 512 : (n + 1) * 512], op[:], recips[qix][:, cc : cc + 1]
        )
    if n == 1:
        nc.sync.dma_start(
            out_ext[qix * QC + cc * 128 : qix * QC + (cc + 1) * 128, :], out_sb[:]
        )


_NC = None


def _get_nc():
    global _NC
    if _NC is None:
        nc = bacc.Bacc("TRN2", target_bir_lowering=False, debug=False,
                       num_devices=N_CORES)
        xt = nc.dram_tensor("xt", [E, S], BF16, kind="ExternalInput").ap()
        wkv = nc.dram_tensor("wkv", [128, ET * 128], BF16, kind="ExternalInput").ap()
        wq = nc.dram_tensor("wq", [128, ET * 128], BF16, kind="ExternalInput").ap()
        bkv = nc.dram_tensor("bkv", [128], F32, kind="ExternalInput").ap()
        bq = nc.dram_tensor("bq", [64], F32, kind="ExternalInput").ap()
        wo = nc.dram_tensor("wo", [128, E], BF16, kind="ExternalInput").ap()
        out_ext = nc.dram_tensor("out", [SH, E], BF16, kind="ExternalOutput").ap()
        with tile.TileContext(nc) as tc:
            _emit(nc, tc, xt, wkv, wq, bkv, bq, wo, out_ext)
        nc.compile()
        _NC = nc
    return _NC


last_results = None
last_tmpdir = None


def kernel(x, W_qkv, b_qkv, W_out, b_out):
    nc = _get_nc()
    bf = ml_dtypes.bfloat16
    x = np.asarray(x, dtype=np.float32)
    Wq = np.asarray(W_qkv, dtype=np.float32)
    b1 = np.asarray(b_qkv, dtype=np.float32)

    wkv = np.empty((128, ET * 128), dtype=bf)
    wq_p = np.zeros((128, ET * 128), dtype=bf)
    for e in range(ET):
        wkv[:, e * 128 : e * 128 + 64] = Wq[e * 128 : (e + 1) * 128, 64:128]
        wkv[:, e * 128 + 64 : (e + 1) * 128] = Wq[e * 128 : (e + 1) * 128, 128:192]
        wq_p[:, e * 128 : e * 128 + 64] = Wq[e * 128 : (e + 1) * 128, 0:64]
    bkv = np.concatenate([b1[64:128], b1[128:192]]).astype(np.float32)
    bq = np.ascontiguousarray(b1[0:64])
    wo = np.zeros((128, E), dtype=bf)
    wo[0:64] = np.asarray(W_out, dtype=np.float32)
    wo[64] = np.asarray(b_out, dtype=np.float32)

    shared = {"wkv": wkv, "wq": wq_p, "bkv": bkv, "bq": bq, "wo": wo}
    in_maps = []
    for c in range(N_CORES):
        b, h = divmod(c, 2)
        xb = x[b]
        xt = np.empty((E, S), dtype=bf)
        xt[:, 0:SH] = xb[h * SH : (h + 1) * SH].T
        xt[:, SH:S] = xb[(1 - h) * SH : (2 - h) * SH].T
        in_maps.append({"xt": xt, **shared})

    import os
    import tempfile
    import time

    tmpdir = os.environ.get("ATTN_TRACE_DIR") or tempfile.mkdtemp(prefix="attn_trace_")
    res = None
    for attempt in range(3):
        try:
            res = run_bass_kernel_spmd(
                nc, in_maps, core_ids=list(range(N_CORES)), tmpdir=tmpdir
            )
            break
        except Exception:
            # transient NRT_EXEC_UNIT_UNRECOVERABLE has been observed on a
            # first attempt; a clean retry recovers
            if attempt == 2:
                raise
            time.sleep(2.0)
    global last_results, last_tmpdir
    last_results = res
    last_tmpdir = tmpdir

    out = np.empty((B, S, E), dtype=np.float32)
    for c in range(N_CORES):
        b, h = divmod(c, 2)
        out[b, h * SH : (h + 1) * SH] = res.results[c]["out"].astype(np.float32)
    return out
